# revision 8
# baseline (speedup 1.0000x reference)
"""EuclideanCodebook (VQ) Trainium2 kernel — 8-core SPMD.

Per-core (data-parallel over tokens, codebook replicated):
  - scores[t,k] = 2*x.e - ||e_k||^2 via TensorE matmuls (argmax == argmin dist)
  - argmax via DVE max8 + max_index reading PSUM
  - quantize rows via indirect DMA gather from embed
  - counts/embed_sum via onehot(fp16) matmuls accumulated in PSUM
  - ReduceScatter(add) over the 8 cores, then EMA update on a K/8 slice

Precision modes for the score matmul:
  "fp16x3": x = xh+xl, e = eh+el in fp16; xh.eh + xh.el + xl.eh (+2-row
            fp16 bias). ~fp32-accurate; 3x the PE work of f32r.
  "f32r":   single-pass float32r (tf32-like); ~1e-4 relative score error.
"""

import sys

sys.path.insert(0, "/opt/trn_rl_repo")

import numpy as np

import concourse.bacc as bacc
import concourse.bass as bass
import concourse.mybir as mybir
import concourse.tile as tile
from concourse.bass_utils import run_bass_kernel_spmd
from concourse.masks import make_identity

F32 = mybir.dt.float32
F32R = mybir.dt.float32r
F16 = mybir.dt.float16
I16 = mybir.dt.int16
I32 = mybir.dt.int32
U32 = mybir.dt.uint32
AF = mybir.ActivationFunctionType
ALU = mybir.AluOpType

N_CORES = 8
B, S, D, K = 8, 8192, 256, 2048
NSH = (B * S) // N_CORES  # tokens per core = 8192
DECAY = 0.99
EPS = 1e-6

PRECISION = "fp16x3"  # or "f32r"


def build(nc: bass.Bass, precision: str = PRECISION):
    TT = NSH // 128  # token tiles per core (64)
    KSL = K // N_CORES  # codes per core for the EMA slice (256)
    DT = D // 128  # d chunks (2)
    KC = K // 128  # 128-wide k chunks (16)
    fp16x3 = precision == "fp16x3"

    x_in = nc.declare_dram_parameter("x", [NSH, D], F32, isOutput=False)
    e_in = nc.declare_dram_parameter("embed", [K, D], F32, isOutput=False)
    cs_in = nc.declare_dram_parameter("cluster_size", [K], F32, isOutput=False)
    cs_sl_in = nc.declare_dram_parameter("cluster_size_sh", [KSL], F32, isOutput=False)
    ea_in = nc.declare_dram_parameter("embed_avg_sh", [KSL, D], F32, isOutput=False)

    q_out = nc.declare_dram_parameter("quantize", [NSH, D], F32, isOutput=True)
    ind_out = nc.declare_dram_parameter("embed_ind", [NSH], I32, isOutput=True)
    ncs_out = nc.declare_dram_parameter("new_cluster_size", [KSL], F32, isOutput=True)
    nea_out = nc.declare_dram_parameter("new_embed_avg", [KSL, D], F32, isOutput=True)
    ne_out = nc.declare_dram_parameter("new_embed", [KSL, D], F32, isOutput=True)

    esum_local = nc.dram_tensor("esum_local", [K, D + 1], F32)
    esum_rs = nc.dram_tensor("esum_rs", [KSL, D + 1], F32)

    with tile.TileContext(nc) as tc:
        with (
            tc.tile_pool(name="const", bufs=1) as const,
            tc.tile_pool(name="big", bufs=1) as big,
        ):
            ident = const.tile([128, 128], F32)
            make_identity(nc, ident)

            # persistent tiles
            if fp16x3:
                ehT = [big.tile([128, K], F16, tag=f"ehT{d}", name=f"ehT{d}") for d in range(DT)]
                elT = [big.tile([128, K], F16, tag=f"elT{d}", name=f"elT{d}") for d in range(DT)]
                xhT = [big.tile([128, NSH], F16, tag=f"xhT{d}", name=f"xhT{d}") for d in range(DT)]
                xlT = [big.tile([128, NSH], F16, tag=f"xlT{d}", name=f"xlT{d}") for d in range(DT)]
                bias2 = const.tile([2, K], F16)
                ones_b = const.tile([2, 128], F16)
            else:
                erT = [big.tile([128, K], F32R, tag=f"erT{d}", name=f"erT{d}") for d in range(DT)]
                xrT = [big.tile([128, NSH], F32R, tag=f"xrT{d}", name=f"xrT{d}") for d in range(DT)]
                b_rowr = const.tile([1, K], F32R)
                ones_b = const.tile([1, 128], F32R)
            x_ext = big.tile([128, TT * (D + 1)], F16, tag="x_ext")
            x_ext3 = x_ext[:].rearrange("p (t e) -> p t e", t=TT)
            idx32_all = big.tile([128, TT], I32, tag="idx32")
            idxf_all = big.tile([128, TT], F32, tag="idxf")
            total_rep = const.tile([128, 1], F32)
            b_row = const.tile([1, K], F32)
            iota_h = [const.tile([128, K // 2], F16, tag=f"iota{h}", name=f"iota{h}") for h in range(2)]

            # ---------------- prologue: codebook prep ----------------
            with (
                tc.tile_pool(name="pro", bufs=3) as pro,
                tc.tile_pool(name="pros", bufs=1) as pros,
                tc.tile_pool(name="pps", bufs=2, space="PSUM") as pps,
                tc.tile_pool(name="pps1", bufs=1, space="PSUM") as pps1,
            ):
                e2cols = pros.tile([128, KC], F32)
                eT_f32 = [pros.tile([128, K], F32, tag=f"eTf{d}", name=f"eTf{d}") for d in range(DT)]

                for i in range(KC):
                    et = pro.tile([128, D], F32, tag="e_load")
                    nc.sync.dma_start(et[:], e_in[i * 128 : (i + 1) * 128, :])
                    sq = pro.tile([128, D], F32, tag="sq")
                    nc.scalar.activation(
                        sq[:], et[:], AF.Square, accum_out=e2cols[:, i : i + 1]
                    )
                    for d in range(DT):
                        tp = pps.tile([128, 128], F32, tag="tp")
                        nc.tensor.transpose(
                            tp[:], et[:, d * 128 : (d + 1) * 128], ident[:]
                        )
                        nc.scalar.activation(
                            eT_f32[d][:, i * 128 : (i + 1) * 128],
                            tp[:],
                            AF.Copy,
                            scale=2.0,
                        )
                if fp16x3:
                    for d in range(DT):
                        nc.scalar.activation(ehT[d][:], eT_f32[d][:], AF.Copy)
                        nc.vector.tensor_tensor(
                            out=elT[d][:], in0=eT_f32[d][:], in1=ehT[d][:],
                            op=ALU.subtract,
                        )
                else:
                    for d in range(DT):
                        nc.vector.tensor_copy(erT[d][:], eT_f32[d][:])

                # bias row: -||e||^2 as [1, K]
                e2t_ps = pps1.tile([128, 128], F32, tag="e2t")
                nc.tensor.transpose(e2t_ps[:KC, :], e2cols[:], ident[:])
                e2row_t = pros.tile([KC, 128], F32)
                nc.scalar.activation(e2row_t[:], e2t_ps[:KC, :], AF.Copy, scale=-1.0)
                for i in range(KC):
                    nc.sync.dma_start(
                        b_row[0:1, i * 128 : (i + 1) * 128], e2row_t[i : i + 1, :]
                    )
                ones_f = pros.tile([2, 128], F32)
                nc.vector.memset(ones_f[:], 1.0)
                if fp16x3:
                    bh_t = pros.tile([1, K], F16)
                    nc.vector.tensor_copy(bh_t[:], b_row[:])
                    bl_t = pros.tile([1, K], F16)
                    nc.vector.tensor_tensor(
                        out=bl_t[:], in0=b_row[:], in1=bh_t[:], op=ALU.subtract
                    )
                    nc.sync.dma_start(bias2[0:1, :], bh_t[:])
                    nc.sync.dma_start(bias2[1:2, :], bl_t[:])
                    ones_f16 = pros.tile([1, 128], F16)
                    nc.vector.tensor_copy(ones_f16[:], ones_f[0:1, :])
                    nc.sync.dma_start(ones_b[0:1, :], ones_f16[:])
                    nc.sync.dma_start(ones_b[1:2, :], ones_f16[:])
                else:
                    nc.vector.tensor_copy(b_rowr[:], b_row[:])
                    nc.vector.tensor_copy(ones_b[:], ones_f[0:1, :])

                # iota rows for the onehot compare (fp16, exact up to 2048)
                for h in range(2):
                    ioti = pros.tile([128, K // 2], I16, tag=f"ioti{h}")
                    nc.gpsimd.iota(
                        ioti[:], pattern=[[1, K // 2]], base=h * (K // 2),
                        channel_multiplier=0,
                    )
                    nc.vector.tensor_copy(iota_h[h][:], ioti[:])

                # total = 0.99*sum(cluster_size) + 0.01*B*S, replicated [128,1]
                cs16 = pros.tile([128, KC], F32)
                nc.sync.dma_start(cs16[:], cs_in[:].rearrange("(g p) -> p g", p=128))
                cssum = pros.tile([128, 1], F32)
                nc.vector.tensor_reduce(
                    out=cssum[:], in_=cs16[:], axis=mybir.AxisListType.X, op=ALU.add
                )
                import concourse.bass_isa as bass_isa

                csrep = pros.tile([128, 1], F32)
                nc.gpsimd.partition_all_reduce(
                    csrep[:], cssum[:], 128, bass_isa.ReduceOp.add
                )
                bias_const = pros.tile([128, 1], F32)
                nc.vector.memset(bias_const[:], (1.0 - DECAY) * float(B * S))
                nc.scalar.activation(
                    total_rep[:], csrep[:], AF.Copy, scale=DECAY,
                )
                nc.vector.tensor_tensor(
                    out=total_rep[:], in0=total_rep[:], in1=bias_const[:], op=ALU.add
                )

            # ---------------- prologue: x load, transpose, split ----------------
            with (
                tc.tile_pool(name="xload", bufs=3) as xload,
                tc.tile_pool(name="xtps", bufs=3, space="PSUM") as xtps,
            ):
                nc.vector.memset(x_ext3[:, :, D : D + 1], 1.0)
                for t in range(TT):
                    xt = xload.tile([128, D], F32, tag="x_t")
                    nc.sync.dma_start(xt[:], x_in[t * 128 : (t + 1) * 128, :])
                    nc.scalar.activation(x_ext3[:, t, 0:D], xt[:], AF.Copy)
                    for d in range(DT):
                        tp = xtps.tile([128, 128], F32, tag="xtp")
                        nc.tensor.transpose(
                            tp[:], xt[:, d * 128 : (d + 1) * 128], ident[:]
                        )
                        tsl = slice(t * 128, (t + 1) * 128)
                        if fp16x3:
                            nc.scalar.activation(xhT[d][:, tsl], tp[:], AF.Copy)
                            nc.vector.tensor_tensor(
                                out=xlT[d][:, tsl], in0=tp[:], in1=xhT[d][:, tsl],
                                op=ALU.subtract,
                            )
                        else:
                            nc.scalar.activation(xrT[d][:, tsl], tp[:], AF.Copy)

            # ---------------- phase A: scores + argmax + gather ----------------
            with (
                tc.tile_pool(name="pa", bufs=3) as pa,
                tc.tile_pool(name="paq", bufs=3) as paq,
                tc.tile_pool(name="scps", bufs=2, space="PSUM") as scps,
            ):
                for t in range(TT):
                    sc = scps.tile([128, K], F32, tag="sc")
                    tsl = slice(t * 128, (t + 1) * 128)
                    if fp16x3:
                        for h in range(4):
                            hs = slice(h * 512, (h + 1) * 512)
                            nc.tensor.matmul(
                                sc[:, hs], ones_b[:], bias2[:, hs],
                                start=True, stop=False,
                            )
                        passes = [(xhT, ehT), (xhT, elT), (xlT, ehT)]
                        for pi, (xs, es) in enumerate(passes):
                            for d in range(DT):
                                for h in range(4):
                                    hs = slice(h * 512, (h + 1) * 512)
                                    nc.tensor.matmul(
                                        sc[:, hs], xs[d][:, tsl], es[d][:, hs],
                                        start=False,
                                        stop=(pi == 2 and d == DT - 1),
                                    )
                    else:
                        for h in range(4):
                            hs = slice(h * 512, (h + 1) * 512)
                            nc.tensor.matmul(
                                sc[:, hs], ones_b[:], b_rowr[0:1, hs],
                                start=True, stop=False,
                            )
                        for d in range(DT):
                            for h in range(4):
                                hs = slice(h * 512, (h + 1) * 512)
                                nc.tensor.matmul(
                                    sc[:, hs], xrT[d][:, tsl], erT[d][:, hs],
                                    start=False, stop=(d == DT - 1),
                                )

                    m8 = pa.tile([128, 8], F32, tag="m8")
                    nc.vector.max(m8[:], sc[:])
                    i8 = pa.tile([128, 8], U32, tag="i8")
                    nc.vector.max_index(i8[:], m8[:], sc[:])
                    nc.vector.tensor_copy(idx32_all[:, t : t + 1], i8[:, 0:1])
                    nc.vector.tensor_copy(idxf_all[:, t : t + 1], i8[:, 0:1])

                    qt = paq.tile([128, D], F32, tag="qt")
                    nc.gpsimd.indirect_dma_start(
                        out=qt[:],
                        out_offset=None,
                        in_=e_in[:],
                        in_offset=bass.IndirectOffsetOnAxis(
                            ap=idx32_all[:, t : t + 1], axis=0
                        ),
                    )
                    nc.sync.dma_start(q_out[t * 128 : (t + 1) * 128, :], qt[:])

                nc.sync.dma_start(
                    ind_out[:].rearrange("(t p) -> p t", p=128), idx32_all[:]
                )

            # ---------------- phase B: onehot matmuls (embed_sum + counts) ----
            with (
                tc.tile_pool(name="pb", bufs=3) as pb,
                tc.tile_pool(name="accps", bufs=1, space="PSUM") as accps,
            ):
                for h in range(2):
                    accs = [
                        accps.tile([128, D + 1], F32, tag=f"acc{m}", name=f"acc{m}")
                        for m in range(8)
                    ]
                    for t in range(TT):
                        oh = pb.tile([128, K // 2], F16, tag="oh")
                        nc.vector.tensor_scalar(
                            out=oh[:],
                            in0=iota_h[h][:],
                            scalar1=idxf_all[:, t : t + 1],
                            scalar2=None,
                            op0=ALU.is_equal,
                        )
                        for m in range(8):
                            nc.tensor.matmul(
                                accs[m][:],
                                oh[:, m * 128 : (m + 1) * 128],
                                x_ext3[:, t, :],
                                start=(t == 0),
                                stop=(t == TT - 1),
                            )
                    for m in range(8):
                        kb = h * 8 + m
                        esb = pb.tile([128, D + 1], F32, tag="esb")
                        nc.scalar.activation(esb[:], accs[m][:], AF.Copy)
                        nc.sync.dma_start(
                            esum_local[kb * 128 : (kb + 1) * 128, :], esb[:]
                        )

            # ---------------- phase C: reduce-scatter + EMA ----------------
            nc.gpsimd.collective_compute(
                "ReduceScatter",
                ALU.add,
                replica_groups=[list(range(N_CORES))],
                ins=[esum_local[:]],
                outs=[esum_rs[:]],
            )

            with tc.tile_pool(name="pc", bufs=1) as pc:
                denom = pc.tile([128, 1], F32, tag="denom")
                nc.vector.tensor_scalar(
                    out=denom[:], in0=total_rep[:], scalar1=float(EPS * K),
                    scalar2=None, op0=ALU.add,
                )
                rden = pc.tile([128, 1], F32, tag="rden")
                nc.vector.reciprocal(rden[:], denom[:])
                factor = pc.tile([128, 1], F32, tag="factor")
                nc.vector.tensor_tensor(
                    out=factor[:], in0=total_rep[:], in1=rden[:], op=ALU.mult
                )

                cs_sl2 = pc.tile([128, KSL // 128], F32, tag="cs_sl")
                nc.sync.dma_start(
                    cs_sl2[:], cs_sl_in[:].rearrange("(j p) -> p j", p=128)
                )
                ncs2 = pc.tile([128, KSL // 128], F32, tag="ncs2")

                for j in range(KSL // 128):
                    es_t = pc.tile([128, D + 1], F32, tag=f"es{j}")
                    nc.sync.dma_start(es_t[:], esum_rs[j * 128 : (j + 1) * 128, :])
                    ea_t = pc.tile([128, D], F32, tag=f"ea{j}")
                    nc.sync.dma_start(ea_t[:], ea_in[j * 128 : (j + 1) * 128, :])

                    ncs_t = ncs2[:, j : j + 1]
                    cnt_s = pc.tile([128, 1], F32, tag=f"cnt{j}")
                    nc.vector.tensor_scalar(
                        out=cnt_s[:], in0=es_t[:, D : D + 1], scalar1=1.0 - DECAY,
                        scalar2=None, op0=ALU.mult,
                    )
                    nc.vector.tensor_scalar(
                        out=ncs_t, in0=cs_sl2[:, j : j + 1], scalar1=DECAY,
                        scalar2=None, op0=ALU.mult,
                    )
                    nc.vector.tensor_tensor(
                        out=ncs_t, in0=ncs_t, in1=cnt_s[:], op=ALU.add
                    )

                    nea_t = pc.tile([128, D], F32, tag=f"nea{j}")
                    nc.vector.tensor_scalar(
                        out=nea_t[:], in0=ea_t[:], scalar1=DECAY, scalar2=None,
                        op0=ALU.mult,
                    )
                    es_s = pc.tile([128, D], F32, tag=f"ess{j}")
                    nc.vector.tensor_scalar(
                        out=es_s[:], in0=es_t[:, 0:D], scalar1=1.0 - DECAY,
                        scalar2=None, op0=ALU.mult,
                    )
                    nc.vector.tensor_tensor(
                        out=nea_t[:], in0=nea_t[:], in1=es_s[:], op=ALU.add
                    )
                    nc.sync.dma_start(nea_out[j * 128 : (j + 1) * 128, :], nea_t[:])

                    sm = pc.tile([128, 1], F32, tag=f"sm{j}")
                    nc.vector.tensor_scalar(
                        out=sm[:], in0=ncs_t, scalar1=float(EPS), scalar2=None,
                        op0=ALU.add,
                    )
                    nc.vector.tensor_tensor(
                        out=sm[:], in0=sm[:], in1=factor[:], op=ALU.mult
                    )
                    rsm = pc.tile([128, 1], F32, tag=f"rsm{j}")
                    nc.vector.reciprocal(rsm[:], sm[:])
                    ne_t = pc.tile([128, D], F32, tag=f"ne{j}")
                    nc.scalar.activation(ne_t[:], nea_t[:], AF.Copy, scale=rsm[:])
                    nc.sync.dma_start(ne_out[j * 128 : (j + 1) * 128, :], ne_t[:])

                nc.sync.dma_start(
                    ncs_out[:].rearrange("(j p) -> p j", p=128), ncs2[:]
                )

    return nc


_NC_CACHE = {}


def _get_nc(precision=PRECISION):
    if precision not in _NC_CACHE:
        nc = bacc.Bacc(
            None, target_bir_lowering=False, debug=False, num_devices=N_CORES
        )
        build(nc, precision)
        nc.finalize()
        _NC_CACHE[precision] = nc
    return _NC_CACHE[precision]


def kernel(x, embed, cluster_size, embed_avg, precision=PRECISION, trace=False):
    x = np.ascontiguousarray(np.asarray(x, dtype=np.float32))
    embed = np.ascontiguousarray(np.asarray(embed, dtype=np.float32))
    cluster_size = np.ascontiguousarray(np.asarray(cluster_size, dtype=np.float32))
    embed_avg = np.ascontiguousarray(np.asarray(embed_avg, dtype=np.float32))

    xf = x.reshape(-1, D)
    KSL = K // N_CORES
    in_maps = []
    for r in range(N_CORES):
        in_maps.append(
            {
                "x": np.ascontiguousarray(xf[r * NSH : (r + 1) * NSH]),
                "embed": embed,
                "cluster_size": cluster_size,
                "cluster_size_sh": np.ascontiguousarray(
                    cluster_size[r * KSL : (r + 1) * KSL]
                ),
                "embed_avg_sh": np.ascontiguousarray(
                    embed_avg[r * KSL : (r + 1) * KSL]
                ),
            }
        )

    nc = _get_nc(precision)
    res = run_bass_kernel_spmd(nc, in_maps, list(range(N_CORES)), trace=trace)
    rs = res.results
    quantize = np.stack([rs[r]["quantize"] for r in range(N_CORES)]).reshape(B, S, D)
    embed_ind = (
        np.stack([rs[r]["embed_ind"] for r in range(N_CORES)])
        .reshape(B, S)
        .astype(np.int32)
    )
    new_cluster_size = np.concatenate(
        [rs[r]["new_cluster_size"] for r in range(N_CORES)]
    )
    new_embed_avg = np.concatenate([rs[r]["new_embed_avg"] for r in range(N_CORES)])
    new_embed = np.concatenate([rs[r]["new_embed"] for r in range(N_CORES)])
    out = (quantize, embed_ind, new_cluster_size, new_embed_avg, new_embed)
    if trace:
        return out, res
    return out
